# revision 1
# baseline (speedup 1.0000x reference)
import sys
import numpy as np

sys.path.insert(0, "/opt/trn_rl_repo")

import concourse.bass as bass  # noqa: E402
import concourse.tile as tile  # noqa: E402
import concourse.mybir as mybir  # noqa: E402
from concourse import bacc, bass_utils  # noqa: E402
from contextlib import ExitStack  # noqa: E402

F32 = mybir.dt.float32
BF16 = mybir.dt.bfloat16
I16 = mybir.dt.int16
U8 = mybir.dt.uint8

B = 2048
IN = 2048
F = 2048
SIX = 6
LUT = 64
NCORES = 8
BLOC = B // NCORES      # 256 rows per core
HALF = 1024             # units per half
NIH = SIX * HALF        # 6144 gather slots per half

_CACHED = {}
TRACE = False
LAST = {}


def _build_kernel():
    nc = bacc.Bacc("TRN2", debug=False)

    x0_d = nc.dram_tensor("x0", [BLOC, IN], F32, kind="ExternalInput").ap()
    r_ds = [nc.dram_tensor(f"rj{l}", [BLOC, 2, HALF, SIX], F32,
                           kind="ExternalInput").ap() for l in range(3)]
    lut_ds = [nc.dram_tensor(f"lutT{l}", [LUT, F], F32,
                             kind="ExternalInput").ap() for l in range(3)]
    idx_ds = [nc.dram_tensor(f"idxw{l}", [128, 2 * (NIH // 16)], I16,
                             kind="ExternalInput").ap() for l in range(3)]
    pat_d = nc.dram_tensor("pat", [128, NIH], BF16, kind="ExternalInput").ap()
    out_d = nc.dram_tensor("out", [BLOC, F], F32, kind="ExternalOutput").ap()

    NW = NIH // 16       # 384 wrapped idx columns per half

    with tile.TileContext(nc) as tc:
        with ExitStack() as ctx:
            cpool = ctx.enter_context(tc.tile_pool(name="const", bufs=1))
            wk = ctx.enter_context(tc.tile_pool(name="wk", bufs=1))
            ps = ctx.enter_context(tc.tile_pool(name="ps", bufs=2, space="PSUM"))

            idxws = [cpool.tile([128, 2 * NW], I16, name=f"idxw{l}")
                     for l in range(3)]
            for l in range(3):
                nc.sync.dma_start(idxws[l][:], idx_ds[l][:])
            ones1 = cpool.tile([1, 128], F32, name="ones1")
            nc.vector.memset(ones1[:], 1.0)
            pat = cpool.tile([128, NIH], BF16, name="pat")
            nc.sync.dma_start(pat[:], pat_d[:])

            # persistent activations (f32), ping-pong across layers
            ybuf = [cpool.tile([128, 2, F], F32, name="yb0"),
                    cpool.tile([128, 2, F], F32, name="yb1")]

            for L in range(3):
                ysrc = ybuf[(L - 1) % 2]
                ydst = ybuf[L % 2]

                e = wk.tile([128, 2, F], BF16, tag="e", name="e")
                for bt in range(2):
                    if L == 0:
                        nc.sync.dma_start(
                            ysrc[:, bt], x0_d[bt * 128:(bt + 1) * 128, :])

                    for h in range(2):
                        xg = wk.tile([128, NIH], F32, tag="xg",
                                     name="xg", bufs=2)
                        nc.gpsimd.ap_gather(
                            xg[:], ysrc[:, bt],
                            idxws[L][:, h * NW:(h + 1) * NW],
                            channels=128, num_elems=F, d=1, num_idxs=NIH)
                        rq = wk.tile([128, NIH], F32, tag="rq",
                                     name="rq")
                        nc.sync.dma_start(
                            rq[:], r_ds[L][bt * 128:(bt + 1) * 128, h])

                        bits = wk.tile([128, NIH], BF16, tag="bits",
                                       name="bits")
                        nc.vector.tensor_tensor(
                            bits[:], xg[:], rq[:], mybir.AluOpType.is_ge)
                        sc = wk.tile([128, NIH], BF16, tag="sc", name="sc")
                        nc.vector.tensor_tensor_scan(
                            sc[:], pat[:], bits[:], 0.0,
                            mybir.AluOpType.mult, mybir.AluOpType.add)
                        nc.vector.tensor_copy(
                            e[:, bt, h * HALF:(h + 1) * HALF], sc[:, 5::6])

                    # per-b-tile 64-way select (lut row broadcast by DMA)
                    for k in range(LUT):
                        lutb = wk.tile([128, F], F32, tag="lutb", name="lutb",
                                       bufs=4)
                        nc.sync.dma_start(
                            lutb[:],
                            lut_ds[L][k:k + 1, :].to_broadcast([128, F]))
                        m = wk.tile([128, F], U8, tag="mask", name="mask",
                                    bufs=2)
                        nc.vector.tensor_scalar(
                            m[:], e[:, bt], float(k), None,
                            mybir.AluOpType.is_equal)
                        nc.vector.copy_predicated(ydst[:, bt], m[:], lutb[:])

                    if L == 2:
                        nc.sync.dma_start(
                            out_d[bt * 128:(bt + 1) * 128, :], ydst[:, bt])


    nc.compile()
    return nc


def _brev6(k):
    r = 0
    for i in range(6):
        r |= ((k >> i) & 1) << (5 - i)
    return r


def _prep_host(lut1, lut2, lut3, connect_1, connect_2, connect_3):
    def sig(x):
        return (1.0 / (1.0 + np.exp(-np.asarray(x, np.float64)))).astype(
            np.float32)

    brev = np.array([_brev6(k) for k in range(LUT)])
    lutTs = [np.ascontiguousarray(sig(lut1)[:, brev].T),
             np.ascontiguousarray(sig(lut2)[:, brev].T),
             np.ascontiguousarray(
                 np.asarray(lut3, np.float32)[:, brev].T)]  # [64, F]

    # ap_gather wrapped idx, slot-major (j fastest) within each half:
    # position i = f''*6 + j -> value connect[1024*h + f'', j]
    idxws = []
    for c in (connect_1, connect_2, connect_3):
        cc = np.asarray(c, np.int64)                       # [F, SIX]
        halves = []
        for h in range(2):
            flat = np.ascontiguousarray(
                cc[h * HALF:(h + 1) * HALF, :]).reshape(NIH)
            wrapped = flat.reshape(NIH // 16, 16).T        # [16, 384]
            halves.append(np.tile(wrapped, (8, 1)))        # [128, 384]
        idxws.append(np.ascontiguousarray(
            np.concatenate(halves, axis=1)).astype(np.int16))

    import ml_dtypes
    pat = np.tile(np.array([0, 2, 2, 2, 2, 2], np.float32), HALF)[None, :]
    pat = np.ascontiguousarray(pat.repeat(128, 0)).astype(ml_dtypes.bfloat16)
    return lutTs, idxws, pat


def _prep_core(inputs, rs, c):
    sl = slice(c * BLOC, (c + 1) * BLOC)
    x0 = ((np.asarray(inputs[sl], np.float32) + np.float32(1.0))
          * np.float32(0.5)).astype(np.float32)
    rjs = []
    for r in rs:
        rr = np.asarray(r[sl], np.float32).reshape(BLOC, 2, HALF, SIX)
        rjs.append(np.ascontiguousarray(rr))
    return x0, rjs


def kernel(inputs, r1, r2, r3, lut1, lut2, lut3,
           connect_1, connect_2, connect_3):
    inputs = np.asarray(inputs, np.float32)
    lutTs, idxws, pat = _prep_host(lut1, lut2, lut3,
                                   connect_1, connect_2, connect_3)

    if "nc" not in _CACHED:
        _CACHED["nc"] = _build_kernel()
    nc = _CACHED["nc"]

    in_maps = []
    for c in range(NCORES):
        x0, rjs = _prep_core(inputs, (r1, r2, r3), c)
        m = {"x0": x0, "pat": pat}
        for l in range(3):
            m[f"rj{l}"] = rjs[l]
            m[f"lutT{l}"] = lutTs[l]
            m[f"idxw{l}"] = idxws[l]
        in_maps.append(m)

    if TRACE:
        import tempfile
        tmpdir = tempfile.mkdtemp(prefix="bass_trace_")
        res = bass_utils.run_bass_kernel_spmd(
            nc, in_maps, core_ids=list(range(NCORES)), trace=True,
            tmpdir=tmpdir)
        LAST["exec_ns"] = res.exec_time_ns
        LAST["trace_dir"] = tmpdir
        LAST["res"] = res
    else:
        res = bass_utils.run_bass_kernel_spmd(
            nc, in_maps, core_ids=list(range(NCORES)))
    out = np.concatenate([res.results[c]["out"] for c in range(NCORES)], axis=0)
    return out.astype(np.float32)

